# revision 14
# baseline (speedup 1.0000x reference)
"""MoE routing layer on 8 Trainium2 NeuronCores (data-parallel over batch).

Per core (4 samples):
  routing MLP -> cosine sim vs embeddings -> softmax weights wf[4,10]
  w_eff[b] = sum_n wf[b,n] * conv_w[n]   (conv is linear in the weights)
  out[b] = conv2d(x[b], w_eff[b]) + b_eff[b]

Conv runs in 64x64 PE tiling mode: 4 independent tiles = (2 samples) x
(2 chunk parities); per pair all 4 PSUM round-banks accumulate together
so one LDWEIGHTS covers 8 matmuls and the tap loop chases the w_eff
slot-group chains produced on VectorE.  x / conv_w / w_eff / out are
fp16 (fp32 accumulation in PSUM).  w_eff uses unnormalized softmax
numerators; the 1/sum(exp) lands in the eviction activation's scale.
"""
import sys

sys.path.insert(0, "/opt/trn_rl_repo")

import numpy as np

import concourse.bass as bass
import concourse.mybir as mybir
from concourse.tile import TileContext

F32 = mybir.dt.float32
F16 = mybir.dt.float16
BF16 = mybir.dt.bfloat16
AF = mybir.ActivationFunctionType
ALU = mybir.AluOpType
AX = mybir.AxisListType

NCORES = 8
BLOC = 4           # samples per core
CIN = 64
COUT = 64
H = W = 58
HW = H * W         # 3364
HWP = HW + 4       # padded flat length
OH = OW = 56
NB = 10            # experts
EDIM = 64
RSIZE = 512
HID = 128
NTAP = 9
CH_ROWS = 7        # output rows per chunk
NCH = 8            # chunks per sample (8*7 = 56)
NFREE = CH_ROWS * W  # 406 <= 512 (one PSUM bank)
TAP_OFF = [dy * W + dx for dy in range(3) for dx in range(3)]
WG = [(0, 3), (3, 6), (6, NTAP)]   # cwp tiles == weff chain slot groups
NWARM_PRE = 5
NWARM_POST = 10

# blob column layout (fp32 columns; fp16 payloads packed 2-per-column)
BL_W1 = 0            # fp16 [128, 512] -> 256 f32 cols
BL_RVT = 256         # fp16 [128, 16]  -> 8
BL_W2 = 264          # fp16 [128, 64]  -> 32
BL_B1 = 296          # f32 [128, 1]
BL_B2 = 297          # f32 [64, 1]
BL_EMB = 298         # f32 [10, 64]
BL_CB2 = 362         # f32 [10, 128]
BL_ID = 490          # f32 [16, 16]
BL_SEL = 506         # f32 [4, 256] -> (j, p)
BL_ONES = 762        # f32 [4, 128]
BLOB_COLS = 890


def fix_sync_waits(nc, cap=2):
    """This walrus build allows at most `cap` sem waits per instruction.
    Splice same-engine NoOps carrying the excess waits right before any
    over-subscribed instruction (waits happen earlier => same semantics)."""
    uid = [0]
    for f in nc.m.functions:
        for blk in f.blocks:
            insts = blk.instructions  # live list
            i = 0
            while i < len(insts):
                inst = insts[i]
                si = inst.sync_info
                waits = list(si.on_wait) if si and si.on_wait else []
                icap = 1
                if len(waits) <= icap:
                    i += 1
                    continue
                keep, excess = waits[-icap:], waits[:-icap]
                for k in range(0, len(excess), icap):
                    nop = mybir.InstNoOp(
                        name=f"{inst.name}-wsplit{uid[0]}", ins=[], outs=[]
                    )
                    uid[0] += 1
                    nop.engine = inst.engine
                    nop.sync_info = mybir.SyncInfo(
                        on_wait=excess[k : k + icap], on_update=[]
                    )
                    nc.register_instruction(nop, overwrite=True)
                    insts.insert(i, nop)
                    i += 1
                inst.sync_info = mybir.SyncInfo(
                    on_wait=keep,
                    on_update=list(si.on_update) if si and si.on_update else [],
                )
                i += 1


def build():
    nc = bass.Bass(num_swdge_queues=4)
    x16 = nc.dram_tensor("x16", [BLOC, CIN, HW], F16, kind="ExternalInput")
    blob = nc.dram_tensor("blob", [128, BLOB_COLS], F32, kind="ExternalInput")
    cwp = nc.dram_tensor("cwp", [128, NB * NTAP * COUT], F16, kind="ExternalInput")
    out16 = nc.dram_tensor(
        "out16", [BLOC, NCH, COUT, CH_ROWS, OW], F16, kind="ExternalOutput"
    )

    with TileContext(nc) as tc:
        with (
            tc.tile_pool(name="consts", bufs=1) as consts,
            tc.tile_pool(name="work", bufs=2) as work,
            tc.tile_pool(name="stage", bufs=2) as stage,
            tc.tile_pool(name="psc", bufs=4, space="PSUM") as psc,
        ):
            # ---------- DMA first: blob + x on scalar ring, cwp on sync ----------
            blobsb = consts.tile([128, BLOB_COLS], F32, tag="blobsb")
            nc.scalar.dma_start(out=blobsb[:], in_=blob[:])

            xt = consts.tile([128, 2, HWP], F16, tag="xt")
            for j in range(2):
                nc.scalar.dma_start(
                    out=xt[:, j, 0:HW],
                    in_=x16[2 * j : 2 * j + 2].rearrange("b c f -> (b c) f"),
                )

            cwg = []
            off = 0
            for gi, (lo, hi) in enumerate(WG):
                sz = NB * (hi - lo) * COUT
                t = consts.tile([128, NB, hi - lo, COUT], F16, name=f"cwg{gi}",
                                tag=f"cwg{gi}")
                nc.sync.dma_start(out=t[:], in_=cwp[:, off : off + sz])
                cwg.append(t)
                off += sz

            # ---------- activation-table preload (hides ~1.3us table load) ----------
            ones64 = consts.tile([EDIM, 1], F32, tag="ones64")
            nc.vector.memset(ones64[:], 1.0)
            nc.vector.memset(xt[:, :, HW:HWP], 0.0)
            tpre = work.tile([EDIM, 1], F32, tag="tpre")
            nc.scalar.activation(out=tpre[:], in_=ones64[:], func=AF.Ln)
            nc.scalar.activation(out=tpre[:], in_=tpre[:], func=AF.Exp)

            # blob views (fp16 payloads live in bitcast columns)
            blob16 = blobsb[:].bitcast(F16)
            w1sb = blob16[:, 0 : 2 * 256].rearrange("p (c m) -> p c m", c=4)
            rvTsb = blob16[:, 2 * BL_RVT : 2 * BL_RVT + 16].rearrange(
                "p (c b) -> p c b", c=4
            )
            w2sb = blob16[:, 2 * BL_W2 : 2 * BL_W2 + EDIM]
            b1sb = blobsb[:, BL_B1 : BL_B1 + 1]
            b2sb = blobsb[0:EDIM, BL_B2 : BL_B2 + 1]
            embsb = blobsb[0:NB, BL_EMB : BL_EMB + EDIM]
            cb2sb = blobsb[0:NB, BL_CB2 : BL_CB2 + 128]
            identsb = blobsb[0:16, BL_ID : BL_ID + 16]
            selsb = blobsb[0:BLOC, BL_SEL : BL_SEL + 256].rearrange(
                "p (j q) -> p j q", j=2
            )
            ones4 = blobsb[0:BLOC, BL_ONES : BL_ONES + 128]

            # ---------- PE warmup helper (keeps HAM clock-gate open) ----------
            warm_src = cwg[0][:].rearrange("p n s c -> p (n s c)")

            def emit_warm(k):
                for _ in range(k):
                    wps = psc.tile([128, NFREE], F32, tag="pA", name="warmps")
                    nc.tensor.matmul(
                        wps[:, 0:384], warm_src[:, 0:128], warm_src[:, 0:384],
                        start=True, stop=True,
                    )

            emit_warm(NWARM_PRE)

            # ---------- routing MLP (fp16 weights, fp32 psum) ----------
            h1ps = psc.tile([HID, BLOC], F32, tag="pA", name="h1ps")
            for c in range(4):
                nc.tensor.matmul(
                    h1ps[:], w1sb[:, c, :], rvTsb[:, c, :],
                    start=(c == 0), stop=(c == 3),
                )
            h1r = work.tile([HID, BLOC], F16, tag="h1r")
            nc.scalar.activation(
                out=h1r[:], in_=h1ps[:], func=AF.Relu, bias=b1sb, scale=1.0
            )
            rps = psc.tile([EDIM, BLOC], F32, tag="pA", name="rps")
            nc.tensor.matmul(rps[:], w2sb, h1r[:], start=True, stop=True)
            rsb = work.tile([EDIM, BLOC], F32, tag="rsb")
            nc.scalar.activation(
                out=rsb[:], in_=rps[:], func=AF.Identity, bias=b2sb, scale=1.0
            )

            # ---------- cosine similarity (emb side runs early, off-path) ----------
            esq = work.tile([NB, EDIM], F32, tag="esq")
            nc.vector.tensor_mul(esq[:], embsb, embsb)
            ensq = work.tile([NB, 1], F32, tag="ensq")
            nc.vector.tensor_reduce(ensq[:], esq[:], axis=AX.X, op=ALU.add)
            eln = work.tile([NB, 1], F32, tag="eln")
            nc.scalar.activation(out=eln[:], in_=ensq[:], func=AF.Ln)
            einv = work.tile([NB, 1], F32, tag="einv")
            nc.scalar.activation(out=einv[:], in_=eln[:], func=AF.Exp, scale=-0.5)
            embn = work.tile([NB, EDIM], F32, tag="embn")
            nc.vector.tensor_scalar_mul(out=embn[:], in0=embsb, scalar1=einv[:])
            embnT_ps = psc.tile([EDIM, NB], F32, tag="pB", name="embnT_ps")
            nc.tensor.transpose(embnT_ps[:], embn[:], identsb[0:NB, 0:NB])
            embnT = work.tile([EDIM, NB], F32, tag="embnT")
            nc.scalar.copy(out=embnT[:], in_=embnT_ps[:])

            rsq = work.tile([EDIM, BLOC], F32, tag="rsq")
            nc.vector.tensor_mul(rsq[:], rsb[:], rsb[:])
            nsq = psc.tile([BLOC, 1], F32, tag="pB", name="nsq")
            nc.tensor.matmul(nsq[:], rsq[:], ones64[:], start=True, stop=True)
            rln = work.tile([BLOC, 1], F32, tag="rln")
            nc.scalar.activation(out=rln[:], in_=nsq[:], func=AF.Ln)
            rinv = work.tile([BLOC, 1], F32, tag="rinv")
            nc.scalar.activation(out=rinv[:], in_=rln[:], func=AF.Exp, scale=-0.5)

            # sim (unscaled) in PSUM; exp(rinv*(sim - mx)) via ACT scale/bias
            simps = psc.tile([BLOC, NB], F32, tag="pB", name="simps")
            nc.tensor.matmul(simps[:], rsb[:], embnT[:], start=True, stop=True)
            mxr = work.tile([BLOC, 1], F32, tag="mxr")
            nc.vector.tensor_reduce(mxr[:], simps[:], axis=AX.X, op=ALU.max)
            negmxr = work.tile([BLOC, 1], F32, tag="negmxr")
            nc.vector.tensor_scalar(
                out=negmxr[:], in0=mxr[:], scalar1=rinv[:], op0=ALU.mult,
                scalar2=-1.0, op1=ALU.mult,
            )
            ex = work.tile([BLOC, NB], F32, tag="ex")
            nc.scalar.activation(
                out=ex[:], in_=simps[:], func=AF.Exp, bias=negmxr[:],
                scale=rinv[:],
            )

            # ---------- weight broadcast from unnormalized numerators ----------
            wfbcs = []
            for j in range(2):
                wfbc_ps = psc.tile([128, NB], F32, tag="pB", name=f"wfbc_ps{j}")
                nc.tensor.matmul(
                    wfbc_ps[:], selsb[:, j, :], ex[:], start=True, stop=True
                )
                wfbc = work.tile([128, NB], F32, tag=f"wfbc{j}")
                nc.scalar.copy(out=wfbc[:], in_=wfbc_ps[:])
                wfbcs.append(wfbc)

            # ---------- w_eff STT chains (fp16, sliced by slot group) ----------
            def weff_chain(weff, wfbc, gi):
                lo, hi = WG[gi]
                nc.vector.tensor_scalar_mul(
                    out=weff[:, lo:hi],
                    in0=cwg[gi][:, 0, :, :],
                    scalar1=wfbc[:, 0:1],
                )
                for n in range(1, NB):
                    nc.vector.scalar_tensor_tensor(
                        out=weff[:, lo:hi],
                        in0=cwg[gi][:, n, :, :],
                        scalar=wfbc[:, n : n + 1],
                        in1=weff[:, lo:hi],
                        op0=ALU.mult,
                        op1=ALU.add,
                    )

            # normalization for eviction bias/scale (tiny, before the chains)
            s = work.tile([BLOC, 1], F32, tag="s")
            nc.vector.tensor_reduce(s[:], ex[:], axis=AX.X, op=ALU.add)
            sinv = work.tile([BLOC, 1], F32, tag="sinv")
            nc.vector.reciprocal(sinv[:], s[:])
            wf = work.tile([BLOC, NB], F32, tag="wf")
            nc.vector.tensor_scalar_mul(out=wf[:], in0=ex[:], scalar1=sinv[:])
            diag4 = work.tile([BLOC, BLOC], F32, tag="diag4")
            nc.vector.tensor_scalar_mul(
                out=diag4[:], in0=identsb[0:BLOC, 0:BLOC], scalar1=sinv[:]
            )

            weffs = [
                work.tile([128, NTAP, COUT], F16, tag=f"weff{j}", name=f"weff{j}")
                for j in range(2)
            ]
            for j in range(2):
                for gi in range(len(WG)):
                    weff_chain(weffs[j], wfbcs[j], gi)

            # bias path (PE + ACT, tiny; runs while weff chains stream)
            wfT_ps = psc.tile([NB, BLOC], F32, tag="pB", name="wfT_ps")
            nc.tensor.transpose(wfT_ps[:], wf[:], identsb[0:BLOC, 0:BLOC])
            wfT = work.tile([NB, BLOC], F32, tag="wfT")
            nc.scalar.copy(out=wfT[:], in_=wfT_ps[:])
            beff_ps = psc.tile([128, BLOC], F32, tag="pB", name="beff_ps")
            nc.tensor.matmul(beff_ps[:], cb2sb, wfT[:], start=True, stop=True)
            beffSB = work.tile([128, BLOC], F32, tag="beffSB")
            nc.scalar.copy(out=beffSB[:], in_=beff_ps[:])
            sinvd_ps = psc.tile([128, BLOC], F32, tag="pB", name="sinvd_ps")
            nc.tensor.matmul(sinvd_ps[:], ones4, diag4[:], start=True, stop=True)
            sinvd = work.tile([128, BLOC], F32, tag="sinvd")
            nc.scalar.copy(out=sinvd[:], in_=sinvd_ps[:])

            emit_warm(NWARM_POST)

            # ---------- conv: 64x64 tiling; all 4 round-banks per pair ----------
            # bank r holds chunks 2r (psum lo half) and 2r+1 (hi half).
            for j in range(2):
                weff = weffs[j]
                pa = [
                    psc.tile([128, NFREE], F32, tag="pA", name=f"pa{j}{r}")
                    for r in range(4)
                ]
                pb = [
                    psc.tile([128, NFREE], F32, tag="pB", name=f"pb{j}{r}")
                    for r in range(4)
                ]
                for t in range(NTAP):
                    first = t == 0
                    last = t == NTAP - 1
                    # (sample half, psum half): one LDW covers 4 rounds
                    for hs, hp in ((0, 0), (0, 1), (1, 0), (1, 1)):
                        wsl = weff[64 * hs : 64 * hs + 64, t, :]
                        xsl = xt[64 * hs : 64 * hs + 64, j]
                        ps = pa if hs == 0 else pb
                        for r in range(4):
                            c = 2 * r + hp
                            o = 7 * c * W + TAP_OFF[t]
                            nc.tensor.matmul(
                                ps[r][64 * hp : 64 * hp + 64],
                                wsl,
                                xsl[:, o : o + NFREE],
                                start=first, stop=last,
                            )
                # evict: out = conv*sinv + beff, fp16; one store per sample
                for s_, banks in ((0, pa), (1, pb)):
                    b = 2 * j + s_
                    st = stage.tile(
                        [128, 4, CH_ROWS, OW], F16, tag="st", name=f"st{j}{s_}"
                    )
                    for r in range(4):
                        psv = banks[r][:].rearrange(
                            "p (h w) -> p h w", w=W
                        )[:, :, 0:OW]
                        nc.scalar.activation(
                            out=st[:, r],
                            in_=psv,
                            func=AF.Identity,
                            bias=beffSB[:, b : b + 1],
                            scale=sinvd[:, b : b + 1],
                        )
                    dst = out16[b].rearrange("(r g2) c h w -> (g2 c) r h w", r=4)
                    eng = nc.sync if s_ == 0 else nc.scalar
                    eng.dma_start(out=dst, in_=st[:])

    fix_sync_waits(nc)
    return nc


_NC = None


def _get_nc():
    global _NC
    if _NC is None:
        _NC = build()
    return _NC


def make_in_maps(inputs):
    x = np.asarray(inputs["x"], dtype=np.float32)
    rvec = np.asarray(inputs["routing_vector"], dtype=np.float32)
    W1 = np.asarray(inputs["W1"], dtype=np.float32)
    b1 = np.asarray(inputs["b1"], dtype=np.float32)
    b2 = np.asarray(inputs["b2"], dtype=np.float32)
    W2 = np.asarray(inputs["W2"], dtype=np.float32)
    emb = np.asarray(inputs["emb"], dtype=np.float32)
    conv_w = np.asarray(inputs["conv_w"], dtype=np.float32)
    conv_b = np.asarray(inputs["conv_b"], dtype=np.float32)

    x16 = np.ascontiguousarray(x.reshape(32, CIN, HW).astype(np.float16))
    # conv_w[n, co, ci, ky, kx] -> [ci, n, slot, co], dup'd over halves,
    # packed slot-group-major to match the contiguous cw tiles
    cwp1 = conv_w.transpose(2, 0, 3, 4, 1).reshape(CIN, NB, NTAP, COUT)
    cwp2 = np.concatenate([cwp1, cwp1], axis=0).astype(np.float16)
    cwp_h = np.ascontiguousarray(
        np.concatenate(
            [cwp2[:, :, lo:hi, :].reshape(128, -1) for lo, hi in WG],
            axis=1,
        )
    )

    def f16cols(a16):
        """view an even-width fp16 array as packed f32 columns"""
        return a16.view(np.float32)

    blob_common = np.zeros((128, BLOB_COLS), np.float32)
    w1h = W1.reshape(4, 128, HID).transpose(1, 0, 2).reshape(128, 512)
    blob_common[:, BL_W1 : BL_W1 + 256] = f16cols(
        np.ascontiguousarray(w1h).astype(np.float16)
    )
    blob_common[:, BL_W2 : BL_W2 + 32] = f16cols(
        np.ascontiguousarray(W2).astype(np.float16)
    )
    blob_common[:, BL_B1] = b1
    blob_common[0:EDIM, BL_B2] = b2
    blob_common[0:NB, BL_EMB : BL_EMB + EDIM] = emb
    blob_common[0:NB, BL_CB2 : BL_CB2 + 128] = np.concatenate(
        [conv_b, conv_b], axis=1
    )
    blob_common[0:16, BL_ID : BL_ID + 16] = np.eye(16, dtype=np.float32)
    selm = np.zeros((BLOC, 2, 128), np.float32)
    for j in range(2):
        selm[2 * j, j, 0:64] = 1.0
        selm[2 * j + 1, j, 64:128] = 1.0
    blob_common[0:BLOC, BL_SEL : BL_SEL + 256] = selm.reshape(BLOC, 256)
    blob_common[0:BLOC, BL_ONES : BL_ONES + 128] = 1.0

    in_maps = []
    for c in range(NCORES):
        blob_h = blob_common.copy()
        rvc = rvec[BLOC * c : BLOC * (c + 1)]
        rvTh = rvc.reshape(BLOC, 4, 128).transpose(2, 1, 0).reshape(128, 16)
        blob_h[:, BL_RVT : BL_RVT + 8] = f16cols(
            np.ascontiguousarray(rvTh).astype(np.float16)
        )
        in_maps.append(
            {
                "x16": np.ascontiguousarray(x16[BLOC * c : BLOC * (c + 1)]),
                "blob": np.ascontiguousarray(blob_h),
                "cwp": cwp_h,
            }
        )
    return in_maps


def kernel(**inputs):
    from concourse.bass_utils import run_bass_kernel_spmd

    nc = _get_nc()
    in_maps = make_in_maps(inputs)
    res = run_bass_kernel_spmd(nc, in_maps, core_ids=list(range(NCORES)))
    outs = []
    for r in res.results:
        o = np.asarray(r["out16"]).astype(np.float32)
        o = o.transpose(0, 2, 1, 3, 4).reshape(BLOC, COUT, OH, OW)
        outs.append(o)
    return np.concatenate(outs, axis=0)
